# revision 1
# baseline (speedup 1.0000x reference)
"""Multi-head attention (B=2, S=2048, D=1024, H=16, d_k=64) on 8 NeuronCores.

Sharding: data-parallel over batch (4 cores per batch element) x tensor-parallel
over heads (4 heads per core).  Each core computes its 256-wide slice of the
Q/K/V projections, attention for its 4 heads, and a partial output projection
(contribution of its head slice to all 1024 output dims).  Host sums the 4
partials per batch element and adds b_O.

Matmuls run in bf16 (fp32 accumulation in PSUM); softmax runs in fp32 on the
scalar engine (exp with the 1/sqrt(d_k) scale folded into the activation's
affine pre-scale).  The softmax denominator is obtained for free by appending
a ones-column to V so the PV matmul also reduces exp-scores over keys.

The attention phase is ACT(exp)-paced, so V-projection tiles are emitted
just-in-time inside attention chunk 0 and the next chunk's Q-projection /
previous chunk's out-projection fill the PE's gaps — keeping the PE busy so
the HAM clock gate stays at full rate.
"""

import sys

sys.path.insert(0, "/opt/trn_rl_repo")

import numpy as np
import ml_dtypes

import concourse.bass as bass  # noqa: F401  (registers types)
import concourse.bacc as bacc
import concourse.mybir as mybir
import concourse.tile as tile
from concourse import bass_utils

BF16 = ml_dtypes.bfloat16

B = 2
S = 2048
D = 1024
N_HEAD = 16
DK = 64
HPC = 4            # heads per core
DPC = HPC * DK     # 256: per-core projection width
VW = DK + 1        # v tile width per head (64 dims + ones column)
SC = 1024          # query-chunk (columns processed per attention pass)
NKT = S // 128     # 16 key tiles
NST = S // 128     # 16 sequence tiles
KD = D // 128      # 8 contraction tiles over D
N_CORES = 8
SCALE = 1.0 / np.sqrt(DK)

# smalls layout (f32, [128, 336]):
#   col 0/1: b_Q slice as two per-partition bias tiles
#   col 2/3: b_K slice
#   col 4..263: b_V broadcast to 128 partitions in v_aug layout (stride VW, 0 at ones cols)
SM_BQ = 0
SM_BK = 2
SM_BV = 4
SM_W = 336

_cached_nc = None


def _build():
    dt = mybir.dt
    f32, bf16 = dt.float32, dt.bfloat16
    AF = mybir.ActivationFunctionType
    ALU = mybir.AluOpType

    nc = bacc.Bacc("TRN2", target_bir_lowering=False, debug=False,
                   num_devices=N_CORES)

    xq_d = nc.dram_tensor("xq", [D, S], bf16, kind="ExternalInput")
    xk_d = nc.dram_tensor("xk", [D, S], bf16, kind="ExternalInput")
    xv_d = nc.dram_tensor("xv", [D, S], bf16, kind="ExternalInput")
    wq_d = nc.dram_tensor("wq", [D, DPC], bf16, kind="ExternalInput")
    wk_d = nc.dram_tensor("wk", [D, DPC], bf16, kind="ExternalInput")
    wv_d = nc.dram_tensor("wv", [D, DPC], bf16, kind="ExternalInput")
    wo_d = nc.dram_tensor("wo", [DPC, D], bf16, kind="ExternalInput")
    sm_d = nc.dram_tensor("smalls", [128, SM_W], f32, kind="ExternalInput")
    pout_d = nc.dram_tensor("pout", [S, D], f32, kind="ExternalOutput")

    with tile.TileContext(nc) as tc:
        with (
            tc.tile_pool(name="sb", bufs=1) as sb,
            tc.tile_pool(name="pts", bufs=7) as pts,
            tc.tile_pool(name="evs", bufs=4) as evs,
            tc.tile_pool(name="rps", bufs=2) as rps,
            tc.tile_pool(name="ppA", bufs=2, space="PSUM") as ppA,
            tc.tile_pool(name="ppB", bufs=1, space="PSUM") as ppB,
            tc.tile_pool(name="ppC", bufs=1, space="PSUM") as ppC,
        ):
            smalls = sb.tile([128, SM_W], f32, tag="smalls", name="smalls")
            nc.sync.dma_start(smalls[:], sm_d[:])

            def load_rows(dram, n_tiles, width, tagbase, eng=None):
                ts = []
                for i in range(n_tiles):
                    t = sb.tile([128, width], bf16, tag=f"{tagbase}{i}",
                                name=f"{tagbase}{i}")
                    (eng or nc.sync).dma_start(t[:],
                                               dram[i * 128:(i + 1) * 128, :])
                    ts.append(t)
                return ts

            # critical path (k/q projections feed the first exps) streams on
            # the sync HWDGE queues; v/wo load concurrently via gpsimd SWDGE.
            wk_t = load_rows(wk_d, KD, DPC, "wk")
            xk_t = load_rows(xk_d, KD, S, "xk")
            wq_t = load_rows(wq_d, KD, DPC, "wq")
            xq_t = load_rows(xq_d, KD, S, "xq")
            wv_t = load_rows(wv_d, KD, DPC, "wv", eng=nc.gpsimd)
            xv_t = load_rows(xv_d, KD, S, "xv", eng=nc.gpsimd)
            wo_t = load_rows(wo_d, 2, D, "wo", eng=nc.gpsimd)

            # kTz[r][p][c]: rows [64p, 64p+64) hold head (2r+p)'s k.T for
            # query... key chunk c, the other 64 rows are zero.  A K=64
            # matmul runs at half the streaming rate of K=128, so QK uses
            # these zero-padded stationary tiles with the full 128-partition
            # qT as moving operand — the zero rows annihilate the other
            # head's contribution.  Per-chunk tiles keep dependency
            # granularity fine so attention starts before all of K is
            # projected.
            kTz = [[[sb.tile([128, SC], bf16, tag=f"kTz{r}{p}{c}",
                             name=f"kTz{r}{p}{c}") for c in range(2)]
                    for p in range(2)] for r in range(2)]
            for r in range(2):
                for c in range(2):
                    nc.gpsimd.memset(kTz[r][0][c][64:128, :], 0.0)
                    nc.gpsimd.memset(kTz[r][1][c][0:64, :], 0.0)
            qT = [[sb.tile([128, SC], bf16, tag=f"qT{r}{c}", name=f"qT{r}{c}")
                   for c in range(2)] for r in range(2)]
            v_t = [sb.tile([128, HPC * VW], bf16, tag=f"v{i}", name=f"v{i}")
                   for i in range(NST)]
            attnT = [sb.tile([128, S], bf16, tag=f"attnT{r}", name=f"attnT{r}")
                     for r in range(2)]

            # ---- K / Q projections: dst.T[j, s] = sum_d W[d, j] * X[d, s] ----
            def gen_proj_qk(w_tiles, x_tiles, dst, bias_col, m, n0, pool,
                            ptag):
                ps = pool.tile([128, SC], f32, tag=ptag,
                               name=f"psp{bias_col}{m}{n0}")
                for k in range(KD):
                    for h2 in range(2):
                        c0 = n0 * SC + h2 * 512
                        nc.tensor.matmul(
                            ps[:, h2 * 512:(h2 + 1) * 512],
                            lhsT=w_tiles[k][:, m * 128:(m + 1) * 128],
                            rhs=x_tiles[k][:, c0:c0 + 512],
                            start=(k == 0), stop=(k == KD - 1))
                        yield
                if dst is None:  # K projection into zero-padded kTz tiles
                    for p in range(2):
                        pr = slice(p * DK, (p + 1) * DK)
                        nc.vector.tensor_scalar_add(
                            kTz[m][p][n0][pr, :], ps[pr, :],
                            smalls[pr, bias_col + m:bias_col + m + 1])
                else:
                    nc.vector.tensor_scalar_add(
                        dst[m][n0][:, :], ps[:, :],
                        smalls[:, bias_col + m:bias_col + m + 1])

            def proj_qk_chunk(*args):
                for _ in gen_proj_qk(*args):
                    pass

            def make_filler(gens, steps_per_call):
                state = list(gens)

                def filler(kt):
                    n = steps_per_call
                    while n > 0 and state:
                        try:
                            next(state[0])
                            n -= 1
                        except StopIteration:
                            state.pop(0)

                def drain():
                    while state:
                        try:
                            next(state[0])
                        except StopIteration:
                            state.pop(0)

                filler.drain = drain
                return filler

            bvv = smalls[:, SM_BV:SM_BV + HPC * VW].rearrange(
                "p (h x) -> p h x", x=VW)[:, :, 0:DK]

            def gen_proj_v():
                for st in range(NST):
                    pv = ppC.tile([128, DPC], f32, tag="C", name=f"pv{st}")
                    for k in range(KD):
                        nc.tensor.matmul(
                            pv[:, :],
                            lhsT=xv_t[k][:, st * 128:(st + 1) * 128],
                            rhs=wv_t[k][:, :],
                            start=(k == 0), stop=(k == KD - 1))
                        yield
                    vv = v_t[st][:].rearrange("p (h x) -> p h x", x=VW)
                    pvv = pv[:].rearrange("p (h e) -> p h e", e=DK)
                    nc.vector.tensor_tensor(vv[:, :, 0:DK], pvv, bvv,
                                            op=ALU.add)
                    nc.vector.memset(vv[:, :, DK:VW], 1.0)
                    yield

            # Attention is emitted as two interleaved streams: the QK+exp
            # stream leads the PV stream by PIPE k-tile positions (across
            # head boundaries), so the ACT exp pipeline never drains while a
            # head's trailing PV / normalize chain completes.
            PIPE = 3

            def emit_qk(heads, p, pt_q):
                hi, kt = divmod(p, NKT)
                n0, h = heads[hi]
                r = h // 2
                ps = ppA.tile([128, SC], f32, tag="A", name=f"ps{n0}{h}{kt}")
                for h2 in range(2):
                    nc.tensor.matmul(
                        ps[:, h2 * 512:(h2 + 1) * 512],
                        lhsT=kTz[r][h % 2][kt // 8][
                            :, (kt % 8) * 128:(kt % 8 + 1) * 128],
                        rhs=qT[r][n0][:, h2 * 512:(h2 + 1) * 512],
                        start=True, stop=True)
                pt = pts.tile([128, SC], bf16, tag="pt", name=f"pt{n0}{h}{kt}")
                nc.scalar.activation(pt[:], ps[:], AF.Exp, scale=float(SCALE))
                pt_q[p] = pt

            def normalize(n0, h, pa):
                q0 = n0 * SC
                r, off = h // 2, (h % 2) * DK
                den = rps.tile([1, SC], f32, tag="den", name=f"den{n0}{h}")
                nc.vector.tensor_copy(den[0:1, :], pa[DK:VW, :])
                rec = rps.tile([1, SC], f32, tag="rec", name=f"rec{n0}{h}")
                nc.vector.reciprocal_approx_fast(rec[0:1, :], den[0:1, :])
                rb = rps.tile([DK, SC], f32, tag="rb", name=f"rb{n0}{h}")
                nc.gpsimd.partition_broadcast(rb[:], rec[0:1, :])
                if off == 0:
                    nc.vector.tensor_tensor(
                        attnT[r][0:DK, q0:q0 + SC],
                        pa[0:DK, :], rb[:, :], op=ALU.mult)
                else:
                    stg = rps.tile([DK, SC], bf16, tag="stg",
                                   name=f"stg{n0}{h}")
                    nc.vector.tensor_tensor(
                        stg[:, :], pa[0:DK, :], rb[:, :], op=ALU.mult)
                    nc.sync.dma_start(
                        attnT[r][off:off + DK, q0:q0 + SC], stg[:, :])

            def attn_pipeline(heads, fillers):
                total = len(heads) * NKT
                pt_q = {}
                pa_cur = [None]

                def emit_pv(p):
                    hi, kt = divmod(p, NKT)
                    n0, h = heads[hi]
                    if kt == 0:
                        pa_cur[0] = ppB.tile([VW, SC], f32, tag="B",
                                             name=f"pa{n0}{h}")
                    f = fillers[hi]
                    if f is not None:
                        f(kt)
                    pa = pa_cur[0]
                    pt = pt_q.pop(p)
                    for h2 in range(2):
                        nc.tensor.matmul(
                            pa[:, h2 * 512:(h2 + 1) * 512],
                            lhsT=v_t[kt][:, h * VW:(h + 1) * VW],
                            rhs=pt[:, h2 * 512:(h2 + 1) * 512],
                            start=(kt == 0), stop=(kt == NKT - 1))
                    if kt == NKT - 1:
                        normalize(n0, h, pa)
                        if f is not None:
                            f.drain()

                for p in range(total + PIPE):
                    if p < total:
                        emit_qk(heads, p, pt_q)
                    if p >= PIPE:
                        emit_pv(p - PIPE)

            def gen_outproj(sts, pool, ptag, use_act):
                for i, st in enumerate(sts):
                    for h2 in range(2):
                        po = pool.tile([128, 512], f32, tag=ptag,
                                       name=f"po{st}{h2}")
                        for jt in range(2):
                            nc.tensor.matmul(
                                po[:, :],
                                lhsT=attnT[jt][:, st * 128:(st + 1) * 128],
                                rhs=wo_t[jt][:, h2 * 512:(h2 + 1) * 512],
                                start=(jt == 0), stop=(jt == 1))
                            yield
                        og = evs.tile([128, 512], f32, tag="og",
                                      name=f"og{st}{h2}")
                        if use_act and h2 == 0:
                            nc.scalar.copy(og[:], po[:])
                        else:
                            nc.vector.tensor_copy(og[:], po[:])
                        deng = nc.sync if h2 == 0 else nc.gpsimd
                        deng.dma_start(
                            pout_d[st * 128:(st + 1) * 128,
                                   h2 * 512:(h2 + 1) * 512], og[:])
                        yield

            # Emission order = scheduling priority.  Attention heads feed the
            # ACT exp stream; remaining projection / out-projection work is
            # smeared into the attention kt-loops as fine-grained PE filler.
            # Head order 0,1,3,2: the last head of each chunk writes attnT
            # directly (no staging DMA on the out-projection critical path).
            proj_qk_chunk(wk_t, xk_t, None, SM_BK, 0, 0, ppA, "A")
            proj_qk_chunk(wq_t, xq_t, qT, SM_BQ, 0, 0, ppA, "A")
            proj_qk_chunk(wk_t, xk_t, None, SM_BK, 0, 1, ppB, "B")

            heads = [(0, 0), (0, 1), (0, 3), (0, 2),
                     (1, 0), (1, 1), (1, 3), (1, 2)]
            fillers = [
                make_filler([gen_proj_v()], 9),
                make_filler([
                    gen_proj_qk(wk_t, xk_t, None, SM_BK, 1, 0, ppC, "C"),
                    gen_proj_qk(wk_t, xk_t, None, SM_BK, 1, 1, ppC, "C"),
                    gen_proj_qk(wq_t, xq_t, qT, SM_BQ, 1, 0, ppC, "C"),
                ], 6),
                make_filler([gen_proj_qk(wq_t, xq_t, qT, SM_BQ, 0, 1,
                                         ppC, "C")], 2),
                make_filler([], 0),
                make_filler([gen_proj_qk(wq_t, xq_t, qT, SM_BQ, 1, 1,
                                         ppC, "C"),
                             gen_outproj((0, 1), ppC, "C", False)], 3),
                make_filler([gen_outproj((2, 3), ppC, "C", False)], 2),
                make_filler([gen_outproj((4, 5), ppC, "C", False)], 2),
                make_filler([gen_outproj((6, 7), ppC, "C", False)], 2),
            ]
            attn_pipeline(heads, fillers)
            for _ in gen_outproj(range(8, 16), ppA, "A", True):
                pass

    nc.compile()
    return nc


def _get_nc():
    global _cached_nc
    if _cached_nc is None:
        _cached_nc = _build()
    return _cached_nc


def _make_in_maps(Q, K, V, W_Q, b_Q, W_K, b_K, W_V, b_V, W_O, b_O):
    in_maps = []
    for c in range(N_CORES):
        b, g = c // 4, c % 4
        hs = slice(g * DPC, (g + 1) * DPC)
        smalls = np.zeros((128, SM_W), np.float32)
        smalls[:, SM_BQ] = b_Q[hs][:128]
        smalls[:, SM_BQ + 1] = b_Q[hs][128:]
        smalls[:, SM_BK] = b_K[hs][:128]
        smalls[:, SM_BK + 1] = b_K[hs][128:]
        bv = np.zeros((HPC, VW), np.float32)
        bv[:, 0:DK] = b_V[hs].reshape(HPC, DK)
        smalls[:, SM_BV:SM_BV + HPC * VW] = bv.reshape(-1)[None, :]
        in_maps.append({
            "xq": np.ascontiguousarray(Q[b].T).astype(BF16),
            "xk": np.ascontiguousarray(K[b].T).astype(BF16),
            "xv": np.ascontiguousarray(V[b].T).astype(BF16),
            "wq": np.ascontiguousarray(W_Q[hs, :].T).astype(BF16),
            "wk": np.ascontiguousarray(W_K[hs, :].T).astype(BF16),
            "wv": np.ascontiguousarray(W_V[hs, :].T).astype(BF16),
            "wo": np.ascontiguousarray(W_O[:, hs].T).astype(BF16),
            "smalls": smalls,
        })
    return in_maps


def _gather(results, b_O):
    out = np.zeros((B, S, D), np.float32)
    for c in range(N_CORES):
        out[c // 4] += results[c]["pout"]
    out += b_O[None, None, :]
    return out


def run(trace=False, **inputs):
    nc = _get_nc()
    in_maps = _make_in_maps(**inputs)
    res = bass_utils.run_bass_kernel_spmd(
        nc, in_maps, core_ids=list(range(N_CORES)), trace=trace)
    return _gather(res.results, np.asarray(inputs["b_O"], np.float32)), res


def kernel(**inputs):
    out, _ = run(trace=False, **inputs)
    return out



# revision 15
# speedup vs baseline: 1.1400x; 1.1400x over previous
"""Multi-head attention (B=2, S=2048, D=1024, H=16, d_k=64) on 8 NeuronCores.

Sharding: data-parallel over batch (4 cores per batch element) x tensor-parallel
over heads (4 heads per core).  Each core computes its 256-wide slice of the
Q/K/V projections, attention for its 4 heads, and a partial output projection
(contribution of its head slice to all 1024 output dims).  Host sums the 4
partials per batch element and adds b_O.

Matmuls run in bf16 (fp32 accumulation in PSUM); softmax runs in fp32 on the
scalar engine (exp with the 1/sqrt(d_k) scale folded into the activation's
affine pre-scale).  The softmax denominator is obtained for free by appending
a ones-column to V so the PV matmul also reduces exp-scores over keys.

QK uses PE row-tiling: a head's two consecutive key-tiles run concurrently as
K=64 matmuls on array row-halves (0,0)/(64,0), fed by partition-swapped copies
of the kT/qT tiles.  This halves QK's PE occupancy versus zero-padded K=128
matmuls, freeing PE slack for the projections and out-projection which are
smeared into the attention kt-loops as fine-grained filler.

All input DMA goes through the sync HWDGE ring in need-order (wk, xk half 0,
wq, xq half 0, wv, xv half 0, xk half 1, xq half 1, xv half 1, wo) so the
first exp fires as early as the memory system allows; the ACT exp stream then
paces the kernel.
"""

import sys

sys.path.insert(0, "/opt/trn_rl_repo")

import numpy as np
import ml_dtypes

import concourse.bass as bass  # noqa: F401  (registers types)
import concourse.bacc as bacc
import concourse.mybir as mybir
import concourse.tile as tile
from concourse import bass_utils

BF16 = ml_dtypes.bfloat16

B = 2
S = 2048
D = 1024
N_HEAD = 16
DK = 64
HPC = 4            # heads per core
DPC = HPC * DK     # 256: per-core projection width
VW = DK + 1        # v tile width per head (64 dims + ones column)
SC = 1024          # query-chunk (columns processed per attention pass)
NKT = S // 128     # 16 key tiles
NST = S // 128     # 16 sequence tiles
KD = D // 128      # 8 contraction tiles over D
N_CORES = 8
SCALE = 1.0 / np.sqrt(DK)

# smalls layout (f32, [128, 336]):
#   col 0/1: b_Q slice as two per-partition bias tiles
#   col 2/3: b_K slice
#   col 4..263: b_V broadcast to 128 partitions in v_aug layout (stride VW, 0 at ones cols)
SM_BQ = 0
SM_BK = 2
SM_BV = 4
SM_W = 336

_cached_nc = None


def _build(dbg=False):
    dt = mybir.dt
    f32, bf16 = dt.float32, dt.bfloat16
    AF = mybir.ActivationFunctionType
    ALU = mybir.AluOpType

    nc = bacc.Bacc("TRN2", target_bir_lowering=False, debug=False,
                   num_devices=N_CORES)
    dbg_d = {}
    if dbg:
        for nm, shp, dty in [("dkt", [128, SC], bf16), ("dkts", [128, SC], bf16),
                             ("dqt", [128, SC], bf16), ("dqts", [128, SC], bf16),
                             ("dpt0", [128, SC], bf16), ("dpt1", [128, SC], bf16),
                             ("dattn", [128, S], bf16), ("dattn1", [128, S], bf16),
                             ("dkt1", [128, SC], bf16), ("dqt1", [128, SC], bf16)]:
            dbg_d[nm] = nc.dram_tensor(nm, shp, dty, kind="ExternalOutput")

    xq_d = nc.dram_tensor("xq", [D, S], bf16, kind="ExternalInput")
    xk_d = nc.dram_tensor("xk", [D, S], bf16, kind="ExternalInput")
    xv_d = nc.dram_tensor("xv", [D, S], bf16, kind="ExternalInput")
    wq_d = nc.dram_tensor("wq", [D, DPC], bf16, kind="ExternalInput")
    wk_d = nc.dram_tensor("wk", [D, DPC], bf16, kind="ExternalInput")
    wv_d = nc.dram_tensor("wv", [D, DPC], bf16, kind="ExternalInput")
    wo_d = nc.dram_tensor("wo", [DPC, D], bf16, kind="ExternalInput")
    sm_d = nc.dram_tensor("smalls", [128, SM_W], f32, kind="ExternalInput")
    pout_d = nc.dram_tensor("pout", [S, D], f32, kind="ExternalOutput")

    with tile.TileContext(nc) as tc:
        with (
            tc.tile_pool(name="sb", bufs=1) as sb,
            tc.tile_pool(name="pts", bufs=9) as pts,
            tc.tile_pool(name="evs", bufs=3) as evs,
            tc.tile_pool(name="rps", bufs=1) as rps,
            tc.tile_pool(name="ppA", bufs=2, space="PSUM") as ppA,
            tc.tile_pool(name="ppB", bufs=1, space="PSUM") as ppB,
            tc.tile_pool(name="ppC", bufs=1, space="PSUM") as ppC,
        ):
            smalls = sb.tile([128, SM_W], f32, tag="smalls", name="smalls")
            nc.sync.dma_start(smalls[:], sm_d[:])

            def alloc_rows(n_tiles, width, tagbase):
                return [sb.tile([128, width], bf16, tag=f"{tagbase}{i}",
                                name=f"{tagbase}{i}")
                        for i in range(n_tiles)]

            def load_rows(ts, dram, cols=None):
                for i, t in enumerate(ts):
                    if cols is None:
                        nc.sync.dma_start(t[:], dram[i * 128:(i + 1) * 128, :])
                    else:
                        nc.sync.dma_start(
                            t[:, cols], dram[i * 128:(i + 1) * 128, cols])

            wk_t = alloc_rows(KD, DPC, "wk")
            xk_t = alloc_rows(KD, S, "xk")
            wq_t = alloc_rows(KD, DPC, "wq")
            xq_t = alloc_rows(KD, S, "xq")
            wv_t = alloc_rows(KD, DPC, "wv")
            xv_t = alloc_rows(KD, S, "xv")
            wo_t = alloc_rows(2, D, "wo")

            c0 = slice(0, SC)
            c1 = slice(SC, S)
            # input DMA in need-order on the sync HWDGE ring (strict FIFO):
            load_rows(wk_t, wk_d)
            load_rows(xk_t, xk_d, c0)
            load_rows(wq_t, wq_d)
            load_rows(xq_t, xq_d, c0)
            load_rows(xk_t, xk_d, c1)
            load_rows(wv_t, wv_d)
            load_rows(xv_t, xv_d, c0)
            load_rows(xq_t, xq_d, c1)
            load_rows(xv_t, xv_d, c1)
            load_rows(wo_t, wo_d)

            # kT[m][c]: [128, SC] bf16; partitions 0-63 hold head 2m's k.T for
            # key chunk c, partitions 64-127 head 2m+1's.  kTs is the
            # partition-swapped copy (halves exchanged) so that either head's
            # stationary k can be sourced at base partition 0 or 64 for PE
            # row-tiled QK.  Same for qT/qTs over query chunks.
            kT = [[sb.tile([128, SC], bf16, tag=f"kT{m}{c}", name=f"kT{m}{c}")
                   for c in range(2)] for m in range(2)]
            kTs = [[sb.tile([128, SC], bf16, tag=f"kTs{m}{c}",
                            name=f"kTs{m}{c}") for c in range(2)]
                   for m in range(2)]
            qT = [[sb.tile([128, SC], bf16, tag=f"qT{m}{c}", name=f"qT{m}{c}")
                   for c in range(2)] for m in range(2)]
            qTs = [[sb.tile([128, SC], bf16, tag=f"qTs{m}{c}",
                            name=f"qTs{m}{c}") for c in range(2)]
                   for m in range(2)]
            v_t = [sb.tile([128, HPC * VW], bf16, tag=f"v{i}", name=f"v{i}")
                   for i in range(NST)]
            attnT = [sb.tile([128, S], bf16, tag=f"attnT{r}", name=f"attnT{r}")
                     for r in range(2)]

            # ---- K / Q projections: dst.T[j, s] = sum_d W[d, j] * X[d, s] ----
            def gen_proj_qk(w_tiles, x_tiles, dst, dsts, bias_col, m, n0,
                            pool, ptag):
                ps = pool.tile([128, SC], f32, tag=ptag,
                               name=f"psp{bias_col}{m}{n0}")
                for k in range(KD):
                    for h2 in range(2):
                        cc = n0 * SC + h2 * 512
                        nc.tensor.matmul(
                            ps[:, h2 * 512:(h2 + 1) * 512],
                            lhsT=w_tiles[k][:, m * 128:(m + 1) * 128],
                            rhs=x_tiles[k][:, cc:cc + 512],
                            start=(k == 0), stop=(k == KD - 1))
                        yield
                nc.vector.tensor_scalar_add(
                    dst[m][n0][:, :], ps[:, :],
                    smalls[:, bias_col + m:bias_col + m + 1])
                yield
                nc.vector.tensor_copy(dsts[m][n0][0:DK, :],
                                      dst[m][n0][DK:128, :])
                nc.vector.tensor_copy(dsts[m][n0][DK:128, :],
                                      dst[m][n0][0:DK, :])

            def proj_qk_chunk(*args):
                for _ in gen_proj_qk(*args):
                    pass

            def make_filler(gens, steps_per_call):
                state = list(gens)

                def filler(kt):
                    n = steps_per_call
                    while n > 0 and state:
                        try:
                            next(state[0])
                            n -= 1
                        except StopIteration:
                            state.pop(0)

                def drain():
                    while state:
                        try:
                            next(state[0])
                        except StopIteration:
                            state.pop(0)

                filler.drain = drain
                return filler

            bvv = smalls[:, SM_BV:SM_BV + HPC * VW].rearrange(
                "p (h x) -> p h x", x=VW)[:, :, 0:DK]

            def gen_proj_v():
                for st in range(NST):
                    pv = ppC.tile([128, DPC], f32, tag="C", name=f"pv{st}")
                    for k in range(KD):
                        nc.tensor.matmul(
                            pv[:, :],
                            lhsT=xv_t[k][:, st * 128:(st + 1) * 128],
                            rhs=wv_t[k][:, :],
                            start=(k == 0), stop=(k == KD - 1))
                        yield
                    vv = v_t[st][:].rearrange("p (h x) -> p h x", x=VW)
                    pvv = pv[:].rearrange("p (h e) -> p h e", e=DK)
                    nc.vector.tensor_tensor(vv[:, :, 0:DK], pvv, bvv,
                                            op=ALU.add)
                    nc.vector.memset(vv[:, :, DK:VW], 1.0)
                    yield

            # Attention: per (head, chunk), 8 kt-pair slots.  Each slot emits
            # the pair's two row-tiled QK matmul pairs (kt even on array rows
            # 0-63, kt odd on rows 64-127, running concurrently) and their two
            # exps; the PV stream lags by PIPE slots so the ACT exp pipeline
            # never drains while a head's trailing PV / normalize completes.
            PIPE = 3

            def emit_qk(heads, p, pt_q):
                hi_, sl = divmod(p, NKT // 2)
                n0, h = heads[hi_]
                r, par = h // 2, h % 2
                pss, ops = [], []
                for half in range(2):
                    kt = 2 * sl + half
                    ck, kk = divmod(kt, 8)
                    ps = ppA.tile([128, SC], f32, tag="A",
                                  name=f"ps{n0}{h}{kt}")
                    pss.append(ps)
                    if half == 0:
                        lhsT = (kT if par == 0 else kTs)[r][ck]
                        rhs = (qT if par == 0 else qTs)[r][n0]
                        pr = slice(0, DK)
                    else:
                        lhsT = (kTs if par == 0 else kT)[r][ck]
                        rhs = (qTs if par == 0 else qT)[r][n0]
                        pr = slice(DK, 128)
                    ops.append((ps, lhsT, rhs, pr, kk))
                # interleave halves so the two K=64 row-tiles (base partition
                # 0 / 64) run concurrently in the PE array
                for h2 in range(2):
                    for half, (ps, lhsT, rhs, pr, kk) in enumerate(ops):
                        nc.tensor.matmul(
                            ps[:, h2 * 512:(h2 + 1) * 512],
                            lhsT=lhsT[pr, kk * 128:(kk + 1) * 128],
                            rhs=rhs[pr, h2 * 512:(h2 + 1) * 512],
                            start=True, stop=True,
                            tile_position=(half * DK, 0))
                for half in range(2):
                    kt = 2 * sl + half
                    pt = pts.tile([128, SC], bf16, tag="pt",
                                  name=f"pt{n0}{h}{kt}")
                    nc.scalar.activation(pt[:], pss[half][:], AF.Exp,
                                         scale=float(SCALE))
                    pt_q[p * 2 + half] = pt
                    if dbg and p == 0:
                        nc.sync.dma_start(
                            dbg_d["dpt0" if half == 0 else "dpt1"][:], pt[:])

            def normalize(n0, h, pa):
                q0 = n0 * SC
                r, off = h // 2, (h % 2) * DK
                den = rps.tile([1, SC], f32, tag="den", name=f"den{n0}{h}")
                nc.vector.tensor_copy(den[0:1, :], pa[DK:VW, :])
                rec = rps.tile([1, SC], f32, tag="rec", name=f"rec{n0}{h}")
                nc.vector.reciprocal_approx_fast(rec[0:1, :], den[0:1, :])
                rb = rps.tile([DK, SC], f32, tag="rb", name=f"rb{n0}{h}")
                nc.gpsimd.partition_broadcast(rb[:], rec[0:1, :])
                if off == 0:
                    nc.vector.tensor_tensor(
                        attnT[r][0:DK, q0:q0 + SC],
                        pa[0:DK, :], rb[:, :], op=ALU.mult)
                else:
                    stg = rps.tile([DK, SC], bf16, tag="stg",
                                   name=f"stg{n0}{h}")
                    nc.vector.tensor_tensor(
                        stg[:, :], pa[0:DK, :], rb[:, :], op=ALU.mult)
                    nc.gpsimd.dma_start(
                        attnT[r][off:off + DK, q0:q0 + SC], stg[:, :])

            def attn_pipeline(heads, fillers):
                total = len(heads) * (NKT // 2)
                pt_q = {}
                pa_cur = [None]

                def emit_pv(p):
                    hi_, sl = divmod(p, NKT // 2)
                    n0, h = heads[hi_]
                    if sl == 0:
                        pa_cur[0] = ppB.tile([VW, SC], f32, tag="B",
                                             name=f"pa{n0}{h}")
                    f = fillers[hi_]
                    if f is not None:
                        f(sl)
                    pa = pa_cur[0]
                    for half in range(2):
                        kt = 2 * sl + half
                        pt = pt_q.pop(p * 2 + half)
                        for h2 in range(2):
                            nc.tensor.matmul(
                                pa[:, h2 * 512:(h2 + 1) * 512],
                                lhsT=v_t[kt][:, h * VW:(h + 1) * VW],
                                rhs=pt[:, h2 * 512:(h2 + 1) * 512],
                                start=(kt == 0), stop=(kt == NKT - 1))
                    if sl == NKT // 2 - 1:
                        normalize(n0, h, pa)
                        if f is not None:
                            f.drain()

                for p in range(total + PIPE):
                    if p < total:
                        emit_qk(heads, p, pt_q)
                    if p >= PIPE:
                        emit_pv(p - PIPE)

            def gen_outproj(sts, pool, ptag, use_act):
                for i, st in enumerate(sts):
                    for h2 in range(2):
                        po = pool.tile([128, 512], f32, tag=ptag,
                                       name=f"po{st}{h2}")
                        for jt in range(2):
                            nc.tensor.matmul(
                                po[:, :],
                                lhsT=attnT[jt][:, st * 128:(st + 1) * 128],
                                rhs=wo_t[jt][:, h2 * 512:(h2 + 1) * 512],
                                start=(jt == 0), stop=(jt == 1))
                            yield
                        og = evs.tile([128, 512], f32, tag="og",
                                      name=f"og{st}{h2}")
                        if use_act and h2 == 0:
                            nc.scalar.copy(og[:], po[:])
                        else:
                            nc.vector.tensor_copy(og[:], po[:])
                        deng = nc.sync if h2 == 0 else nc.gpsimd
                        deng.dma_start(
                            pout_d[st * 128:(st + 1) * 128,
                                   h2 * 512:(h2 + 1) * 512], og[:])
                        yield

            # Emission order = scheduling priority.  Attention heads feed the
            # ACT exp stream; remaining projection / out-projection work is
            # smeared into the attention kt-loops as fine-grained PE filler.
            # Head order 0,1,3,2: the last head of each chunk writes attnT
            # directly (no staging DMA on the out-projection critical path).
            proj_qk_chunk(wk_t, xk_t, kT, kTs, SM_BK, 0, 0, ppA, "A")
            proj_qk_chunk(wq_t, xq_t, qT, qTs, SM_BQ, 0, 0, ppA, "A")
            proj_qk_chunk(wk_t, xk_t, kT, kTs, SM_BK, 0, 1, ppB, "B")

            heads = [(0, 0), (0, 1), (0, 3), (0, 2),
                     (1, 0), (1, 1), (1, 3), (1, 2)]
            fillers = [
                make_filler([gen_proj_v()], 18),
                make_filler([
                    gen_proj_qk(wk_t, xk_t, kT, kTs, SM_BK, 1, 0, ppC, "C"),
                    gen_proj_qk(wk_t, xk_t, kT, kTs, SM_BK, 1, 1, ppC, "C"),
                    gen_proj_qk(wq_t, xq_t, qT, qTs, SM_BQ, 1, 0, ppC, "C"),
                ], 14),
                make_filler([gen_proj_qk(wq_t, xq_t, qT, qTs, SM_BQ, 0, 1,
                                         ppC, "C")], 5),
                make_filler([], 0),
                make_filler([gen_proj_qk(wq_t, xq_t, qT, qTs, SM_BQ, 1, 1,
                                         ppC, "C"),
                             gen_outproj((0, 1), ppC, "C", False)], 7),
                make_filler([gen_outproj((2, 3), ppC, "C", False)], 4),
                make_filler([gen_outproj((4, 5), ppC, "C", False)], 4),
                make_filler([gen_outproj((6, 7), ppC, "C", False)], 4),
            ]
            attn_pipeline(heads, fillers)
            for _ in gen_outproj(range(8, 16), ppA, "A", True):
                pass
            if dbg:
                nc.sync.dma_start(dbg_d["dkt"][:], kT[0][0][:])
                nc.sync.dma_start(dbg_d["dkts"][:], kTs[0][0][:])
                nc.sync.dma_start(dbg_d["dqt"][:], qT[0][0][:])
                nc.sync.dma_start(dbg_d["dqts"][:], qTs[0][0][:])
                nc.sync.dma_start(dbg_d["dattn"][:], attnT[0][:])
                nc.sync.dma_start(dbg_d["dattn1"][:], attnT[1][:])
                nc.sync.dma_start(dbg_d["dkt1"][:], kT[1][0][:])
                nc.sync.dma_start(dbg_d["dqt1"][:], qT[1][0][:])

    nc.compile()
    return nc


def _get_nc():
    global _cached_nc
    if _cached_nc is None:
        _cached_nc = _build()
    return _cached_nc


def _make_in_maps(Q, K, V, W_Q, b_Q, W_K, b_K, W_V, b_V, W_O, b_O):
    in_maps = []
    for c in range(N_CORES):
        b, g = c // 4, c % 4
        hs = slice(g * DPC, (g + 1) * DPC)
        smalls = np.zeros((128, SM_W), np.float32)
        smalls[:, SM_BQ] = b_Q[hs][:128]
        smalls[:, SM_BQ + 1] = b_Q[hs][128:]
        smalls[:, SM_BK] = b_K[hs][:128]
        smalls[:, SM_BK + 1] = b_K[hs][128:]
        bv = np.zeros((HPC, VW), np.float32)
        bv[:, 0:DK] = b_V[hs].reshape(HPC, DK)
        smalls[:, SM_BV:SM_BV + HPC * VW] = bv.reshape(-1)[None, :]
        in_maps.append({
            "xq": np.ascontiguousarray(Q[b].T).astype(BF16),
            "xk": np.ascontiguousarray(K[b].T).astype(BF16),
            "xv": np.ascontiguousarray(V[b].T).astype(BF16),
            "wq": np.ascontiguousarray(W_Q[hs, :].T).astype(BF16),
            "wk": np.ascontiguousarray(W_K[hs, :].T).astype(BF16),
            "wv": np.ascontiguousarray(W_V[hs, :].T).astype(BF16),
            "wo": np.ascontiguousarray(W_O[:, hs].T).astype(BF16),
            "smalls": smalls,
        })
    return in_maps


def _gather(results, b_O):
    out = np.zeros((B, S, D), np.float32)
    for c in range(N_CORES):
        out[c // 4] += results[c]["pout"]
    out += b_O[None, None, :]
    return out


def run(trace=False, **inputs):
    nc = _get_nc()
    in_maps = _make_in_maps(**inputs)
    res = bass_utils.run_bass_kernel_spmd(
        nc, in_maps, core_ids=list(range(N_CORES)), trace=trace)
    return _gather(res.results, np.asarray(inputs["b_O"], np.float32)), res


def kernel(**inputs):
    out, _ = run(trace=False, **inputs)
    return out


# revision 23
# speedup vs baseline: 1.2777x; 1.1208x over previous
"""Multi-head attention (B=2, S=2048, D=1024, H=16, d_k=64) on 8 NeuronCores.

Sharding: data-parallel over batch (4 cores per batch element) x tensor-parallel
over heads (4 heads per core).  Each core computes its 256-wide slice of the
Q/K/V projections, attention for its 4 heads, and a partial output projection
(contribution of its head slice to all 1024 output dims).  Host sums the 4
partials per batch element and adds b_O.

Matmuls run in bf16 (fp32 accumulation in PSUM); softmax runs in fp32 on the
scalar engine (exp with the 1/sqrt(d_k) scale folded into the activation's
affine pre-scale).  The softmax denominator comes for free from 64 ones
columns appended to each head's V stationary (even heads [v|ones], odd heads
[ones|v]), so the PV matmul fills half its PSUM tile with the denominator
replicated across 64 partitions.  Normalization is then just a reciprocal and
a multiply at full DVE width, with no partition broadcast and no staging DMA:
the odd-head parity swap puts each head's attention rows at the partition
offset where attnT wants them.

All input DMA runs on the sync HWDGE ring in need-order, with x-tiles split
into column halves and interleaved with their weight tiles so the projections
start as soon as the first k-tile lands and track the DMA stream.  The ACT
exp stream paces the kernel; projections and out-projection fill PE gaps.
"""

import sys

sys.path.insert(0, "/opt/trn_rl_repo")

import numpy as np
import ml_dtypes

import concourse.bass as bass  # noqa: F401  (registers types)
import concourse.bacc as bacc
import concourse.mybir as mybir
import concourse.tile as tile
from concourse import bass_utils

BF16 = ml_dtypes.bfloat16

B = 2
S = 2048
D = 1024
N_HEAD = 16
DK = 64
HPC = 4            # heads per core
DPC = HPC * DK     # 256: per-core projection width
VW = 2 * DK        # v tile width per head (64 dims + 64 ones columns)
SC = 1024          # query-chunk (columns processed per attention pass)
NKT = S // 128     # 16 key tiles
NST = S // 128     # 16 sequence tiles
KD = D // 128      # 8 contraction tiles over D
N_CORES = 8
SCALE = 1.0 / np.sqrt(DK)

# smalls layout (f32, [128, 260]):
#   col 0/1: b_Q slice as two per-partition bias tiles
#   col 2/3: b_K slice
#   col 4..259: b_V as [128, 4, 64] (per head h: partitions x dims)
SM_BQ = 0
SM_BK = 2
SM_BV = 4
SM_W = 260

_cached_nc = None


def _build(dbg=False):
    dt = mybir.dt
    f32, bf16 = dt.float32, dt.bfloat16
    AF = mybir.ActivationFunctionType
    ALU = mybir.AluOpType

    nc = bacc.Bacc("TRN2", target_bir_lowering=False, debug=False,
                   num_devices=N_CORES)
    dbg_d = {}
    if dbg:
        for nm, shp in [("dv0", [128, HPC * VW]), ("dv1", [128, HPC * VW]),
                        ("dattn0", [128, S]), ("dattn1", [128, S])]:
            dbg_d[nm] = nc.dram_tensor(nm, shp, bf16, kind="ExternalOutput")

    xq_d = nc.dram_tensor("xq", [D, S], bf16, kind="ExternalInput")
    xk_d = nc.dram_tensor("xk", [D, S], bf16, kind="ExternalInput")
    xv_d = nc.dram_tensor("xv", [D, S], bf16, kind="ExternalInput")
    wq_d = nc.dram_tensor("wq", [D, DPC], bf16, kind="ExternalInput")
    wk_d = nc.dram_tensor("wk", [D, DPC], bf16, kind="ExternalInput")
    wv_d = nc.dram_tensor("wv", [D, DPC], bf16, kind="ExternalInput")
    wo_d = nc.dram_tensor("wo", [DPC, D], bf16, kind="ExternalInput")
    sm_d = nc.dram_tensor("smalls", [128, SM_W], f32, kind="ExternalInput")
    pout_d = nc.dram_tensor("pout", [S, D], f32, kind="ExternalOutput")

    with tile.TileContext(nc) as tc:
        with (
            tc.tile_pool(name="sb", bufs=1) as sb,
            tc.tile_pool(name="pts", bufs=8) as pts,
            tc.tile_pool(name="evs", bufs=4) as evs,
            tc.tile_pool(name="rps", bufs=1) as rps,
            tc.tile_pool(name="ppA", bufs=2, space="PSUM") as ppA,
            tc.tile_pool(name="ppB", bufs=1, space="PSUM") as ppB,
            tc.tile_pool(name="ppC", bufs=1, space="PSUM") as ppC,
        ):
            smalls = sb.tile([128, SM_W], f32, tag="smalls", name="smalls")
            nc.sync.dma_start(smalls[:], sm_d[:])

            def alloc_rows(n_tiles, width, tagbase):
                return [sb.tile([128, width], bf16, tag=f"{tagbase}{i}",
                                name=f"{tagbase}{i}")
                        for i in range(n_tiles)]

            def load_tile(t, dram, i, cols=None):
                if cols is None:
                    nc.sync.dma_start(t[:], dram[i * 128:(i + 1) * 128, :])
                else:
                    nc.sync.dma_start(
                        t[:, cols], dram[i * 128:(i + 1) * 128, cols])

            wk_t = alloc_rows(KD, DPC, "wk")
            xk_t = alloc_rows(KD, S, "xk")
            wq_t = alloc_rows(KD, DPC, "wq")
            xq_t = alloc_rows(KD, S, "xq")
            wv_t = alloc_rows(KD, DPC, "wv")
            xv_t = alloc_rows(KD, S, "xv")
            wo_t = alloc_rows(2, D, "wo")

            c0 = slice(0, SC)
            c1 = slice(SC, S)
            # input DMA in need-order on the sync HWDGE ring (strict FIFO);
            # x k-tiles interleaved with their weight tiles so projection
            # k-loop i can fire as soon as pair i lands.
            for i in range(KD):
                load_tile(xk_t[i], xk_d, i, c0)
                load_tile(wk_t[i], wk_d, i)
            for i in range(KD):
                load_tile(xq_t[i], xq_d, i, c0)
                load_tile(wq_t[i], wq_d, i)
            for i in range(KD):
                load_tile(xk_t[i], xk_d, i, c1)
            for i in range(KD):
                load_tile(xv_t[i], xv_d, i, c0)
                load_tile(wv_t[i], wv_d, i)
            for i in range(KD):
                load_tile(xq_t[i], xq_d, i, c1)
            for i in range(KD):
                load_tile(xv_t[i], xv_d, i, c1)
            for i in range(2):
                load_tile(wo_t[i], wo_d, i)

            # kTz[r][p][c]: rows [64p, 64p+64) hold head (2r+p)'s k.T for key
            # chunk c, the other 64 rows are zero.  QK uses these zero-padded
            # stationary tiles with the full 128-partition qT as moving
            # operand — the zero rows annihilate the other head's
            # contribution, keeping every matmul in plain 128x128 array mode
            # (no tiling-mode switches, which cost a PE drain each way).
            kTz = [[[sb.tile([128, SC], bf16, tag=f"kTz{r}{p}{c}",
                             name=f"kTz{r}{p}{c}") for c in range(2)]
                    for p in range(2)] for r in range(2)]
            for r in range(2):
                for c in range(2):
                    nc.gpsimd.memset(kTz[r][0][c][64:128, :], 0.0)
                    nc.gpsimd.memset(kTz[r][1][c][0:64, :], 0.0)
            qT = [[sb.tile([128, SC], bf16, tag=f"qT{r}{c}", name=f"qT{r}{c}")
                   for c in range(2)] for r in range(2)]
            v_t = [sb.tile([128, HPC * VW], bf16, tag=f"v{i}", name=f"v{i}")
                   for i in range(NST)]
            attnT = [sb.tile([128, S], bf16, tag=f"attnT{r}", name=f"attnT{r}")
                     for r in range(2)]

            # ---- K / Q projections: dst.T[j, s] = sum_d W[d, j] * X[d, s] ----
            def gen_proj_qk(w_tiles, x_tiles, dst, bias_col, m, n0, pool,
                            ptag):
                ps = pool.tile([128, SC], f32, tag=ptag,
                               name=f"psp{bias_col}{m}{n0}")
                for k in range(KD):
                    for h2 in range(2):
                        cc = n0 * SC + h2 * 512
                        nc.tensor.matmul(
                            ps[:, h2 * 512:(h2 + 1) * 512],
                            lhsT=w_tiles[k][:, m * 128:(m + 1) * 128],
                            rhs=x_tiles[k][:, cc:cc + 512],
                            start=(k == 0), stop=(k == KD - 1))
                        yield
                if dst is None:  # K projection into zero-padded kTz tiles
                    for p in range(2):
                        pr = slice(p * DK, (p + 1) * DK)
                        nc.vector.tensor_scalar_add(
                            kTz[m][p][n0][pr, :], ps[pr, :],
                            smalls[pr, bias_col + m:bias_col + m + 1])
                else:
                    nc.vector.tensor_scalar_add(
                        dst[m][n0][:, :], ps[:, :],
                        smalls[:, bias_col + m:bias_col + m + 1])

            def proj_qk_chunk(*args):
                for _ in gen_proj_qk(*args):
                    pass

            def make_filler(gens, steps_per_call):
                state = list(gens)

                def filler(kt):
                    n = steps_per_call
                    while n > 0 and state:
                        try:
                            next(state[0])
                            n -= 1
                        except StopIteration:
                            state.pop(0)

                def drain():
                    while state:
                        try:
                            next(state[0])
                        except StopIteration:
                            state.pop(0)

                filler.drain = drain
                return filler

            bvv = smalls[:, SM_BV:SM_BV + HPC * DK].rearrange(
                "p (h x) -> p h x", x=DK)

            def gen_proj_v():
                # v_aug per head h: [v | 64 ones columns] so PV puts the
                # attention rows at partitions 0-63 and the softmax
                # denominator replicated across partitions 64-127.
                for st in range(NST):
                    pv = ppC.tile([128, DPC], f32, tag="C", name=f"pv{st}")
                    for k in range(KD):
                        nc.tensor.matmul(
                            pv[:, :],
                            lhsT=xv_t[k][:, st * 128:(st + 1) * 128],
                            rhs=wv_t[k][:, :],
                            start=(k == 0), stop=(k == KD - 1))
                        yield
                    vv = v_t[st][:].rearrange("p (h x) -> p h x", x=VW)
                    pvv = pv[:].rearrange("p (h e) -> p h e", e=DK)
                    nc.vector.tensor_tensor(vv[:, :, 0:DK], pvv, bvv,
                                            op=ALU.add)
                    nc.vector.memset(vv[:, :, DK:VW], 1.0)
                    yield

            # The attention phase is ACT(exp)-paced: the QK+exp stream leads
            # the PV stream by PIPE kt positions (across head boundaries), so
            # the ACT exp pipeline never drains while a head's trailing PV /
            # normalize chain completes.
            PIPE = 3

            def emit_qk(heads, p, pt_q):
                hi, kt = divmod(p, NKT)
                n0, h = heads[hi]
                r = h // 2
                ps = ppA.tile([128, SC], f32, tag="A", name=f"ps{n0}{h}{kt}")
                for h2 in range(2):
                    nc.tensor.matmul(
                        ps[:, h2 * 512:(h2 + 1) * 512],
                        lhsT=kTz[r][h % 2][kt // 8][
                            :, (kt % 8) * 128:(kt % 8 + 1) * 128],
                        rhs=qT[r][n0][:, h2 * 512:(h2 + 1) * 512],
                        start=True, stop=True)
                pt = pts.tile([128, SC], bf16, tag="pt", name=f"pt{n0}{h}{kt}")
                nc.scalar.activation(pt[:], ps[:], AF.Exp, scale=float(SCALE))
                pt_q[p] = pt

            def normalize(n0, h, pa):
                q0 = n0 * SC
                r, off = h // 2, (h % 2) * DK
                dn = rps.tile([DK, SC], f32, tag="dn", name=f"dn{n0}{h}")
                rb = rps.tile([DK, SC], f32, tag="rb", name=f"rb{n0}{h}")
                nc.vector.tensor_copy(dn[:, :], pa[DK:128, :])
                nc.vector.reciprocal_approx_fast(rb[:, :], dn[:, :])
                if off == 0:
                    for hh in range(2):
                        cs = slice(hh * 512, (hh + 1) * 512)
                        nc.vector.tensor_tensor(
                            attnT[r][0:DK, q0 + hh * 512:q0 + (hh + 1) * 512],
                            pa[0:DK, cs], rb[:, cs], op=ALU.mult)
                else:
                    stg = rps.tile([DK, SC], bf16, tag="stg",
                                   name=f"stg{n0}{h}")
                    nc.vector.tensor_tensor(stg[:, :], pa[0:DK, :], rb[:, :],
                                            op=ALU.mult)
                    nc.gpsimd.dma_start(
                        attnT[r][off:off + DK, q0:q0 + SC], stg[:, :])

            def attn_pipeline(heads, fillers):
                total = len(heads) * NKT
                pt_q = {}
                pa_cur = [None]

                def emit_pv(p):
                    hi, kt = divmod(p, NKT)
                    n0, h = heads[hi]
                    if kt == 0:
                        pa_cur[0] = ppB.tile([128, SC], f32, tag="B",
                                             name=f"pa{n0}{h}")
                    f = fillers[hi]
                    if f is not None:
                        f(kt)
                    pa = pa_cur[0]
                    pt = pt_q.pop(p)
                    for h2 in range(2):
                        nc.tensor.matmul(
                            pa[:, h2 * 512:(h2 + 1) * 512],
                            lhsT=v_t[kt][:, h * VW:(h + 1) * VW],
                            rhs=pt[:, h2 * 512:(h2 + 1) * 512],
                            start=(kt == 0), stop=(kt == NKT - 1))
                    if kt == NKT - 1:
                        normalize(n0, h, pa)
                        if f is not None:
                            f.drain()

                for p in range(total + PIPE):
                    if p < total:
                        emit_qk(heads, p, pt_q)
                    if p >= PIPE:
                        emit_pv(p - PIPE)

            def gen_outproj(sts, pool, ptag, use_act):
                for i, st in enumerate(sts):
                    for h2 in range(2):
                        po = pool.tile([128, 512], f32, tag=ptag,
                                       name=f"po{st}{h2}")
                        for jt in range(2):
                            nc.tensor.matmul(
                                po[:, :],
                                lhsT=attnT[jt][:, st * 128:(st + 1) * 128],
                                rhs=wo_t[jt][:, h2 * 512:(h2 + 1) * 512],
                                start=(jt == 0), stop=(jt == 1))
                            yield
                        og = evs.tile([128, 512], f32, tag="og",
                                      name=f"og{st}{h2}")
                        if use_act and h2 == 0:
                            nc.scalar.copy(og[:], po[:])
                        else:
                            nc.vector.tensor_copy(og[:], po[:])
                        deng = nc.sync if h2 == 0 else nc.gpsimd
                        deng.dma_start(
                            pout_d[st * 128:(st + 1) * 128,
                                   h2 * 512:(h2 + 1) * 512], og[:])
                        yield

            def interleave(*gens):
                gens = list(gens)
                while gens:
                    g = gens.pop(0)
                    try:
                        next(g)
                        gens.append(g)
                    except StopIteration:
                        pass

            # Emission order = scheduling priority.  Attention heads feed the
            # ACT exp stream; remaining projection / out-projection work is
            # smeared into the attention kt-loops as fine-grained PE filler.
            # Head order 0,1,3,2: each chunk ends on an even head (direct
            # attnT write at partition 0) so the out-projection's last
            # dependency is produced with the shortest normalize chain.
            proj_qk_chunk(wk_t, xk_t, None, SM_BK, 0, 0, ppA, "A")
            proj_qk_chunk(wq_t, xq_t, qT, SM_BQ, 0, 0, ppA, "A")
            proj_qk_chunk(wk_t, xk_t, None, SM_BK, 0, 1, ppB, "B")

            heads = [(0, 0), (0, 1), (0, 3), (0, 2),
                     (1, 0), (1, 1), (1, 3), (1, 2)]
            fillers = [
                make_filler([gen_proj_v()], 11),
                make_filler([
                    gen_proj_qk(wk_t, xk_t, None, SM_BK, 1, 0, ppC, "C"),
                    gen_proj_qk(wk_t, xk_t, None, SM_BK, 1, 1, ppC, "C"),
                    gen_proj_qk(wq_t, xq_t, qT, SM_BQ, 1, 0, ppC, "C"),
                ], 6),
                make_filler([gen_proj_qk(wq_t, xq_t, qT, SM_BQ, 0, 1,
                                         ppC, "C")], 2),
                make_filler([], 0),
                make_filler([gen_proj_qk(wq_t, xq_t, qT, SM_BQ, 1, 1,
                                         ppC, "C"),
                             gen_outproj((0, 1), ppC, "C", False)], 3),
                make_filler([gen_outproj((2, 3), ppC, "C", False)], 2),
                make_filler([gen_outproj((4, 5), ppC, "C", False),
                             gen_outproj((6, 7), ppC, "C", False)], 4),
                make_filler([], 0),
            ]
            attn_pipeline(heads, fillers)
            # tail out-projection: two chains on separate PSUM pools so the
            # po->og->DMA pipelines overlap instead of serializing on slots
            interleave(gen_outproj((8, 10, 12, 14), ppA, "A", True),
                       gen_outproj((9, 11, 13, 15), ppC, "C", True))
            if dbg:
                nc.sync.dma_start(dbg_d["dv0"][:], v_t[0][:])
                nc.sync.dma_start(dbg_d["dv1"][:], v_t[1][:])
                nc.sync.dma_start(dbg_d["dattn0"][:], attnT[0][:])
                nc.sync.dma_start(dbg_d["dattn1"][:], attnT[1][:])

    nc.compile()
    return nc


def _get_nc():
    global _cached_nc
    if _cached_nc is None:
        _cached_nc = _build()
    return _cached_nc


def _make_in_maps(Q, K, V, W_Q, b_Q, W_K, b_K, W_V, b_V, W_O, b_O):
    in_maps = []
    for c in range(N_CORES):
        b, g = c // 4, c % 4
        hs = slice(g * DPC, (g + 1) * DPC)
        smalls = np.zeros((128, SM_W), np.float32)
        smalls[:, SM_BQ] = b_Q[hs][:128]
        smalls[:, SM_BQ + 1] = b_Q[hs][128:]
        smalls[:, SM_BK] = b_K[hs][:128]
        smalls[:, SM_BK + 1] = b_K[hs][128:]
        smalls[:, SM_BV:SM_BV + HPC * DK] = b_V[hs].reshape(-1)[None, :]
        in_maps.append({
            "xq": np.ascontiguousarray(Q[b].T).astype(BF16),
            "xk": np.ascontiguousarray(K[b].T).astype(BF16),
            "xv": np.ascontiguousarray(V[b].T).astype(BF16),
            "wq": np.ascontiguousarray(W_Q[hs, :].T).astype(BF16),
            "wk": np.ascontiguousarray(W_K[hs, :].T).astype(BF16),
            "wv": np.ascontiguousarray(W_V[hs, :].T).astype(BF16),
            "wo": np.ascontiguousarray(W_O[:, hs].T).astype(BF16),
            "smalls": smalls,
        })
    return in_maps


def _gather(results, b_O):
    out = np.zeros((B, S, D), np.float32)
    for c in range(N_CORES):
        out[c // 4] += results[c]["pout"]
    out += b_O[None, None, :]
    return out


def run(trace=False, **inputs):
    nc = _get_nc()
    in_maps = _make_in_maps(**inputs)
    res = bass_utils.run_bass_kernel_spmd(
        nc, in_maps, core_ids=list(range(N_CORES)), trace=trace)
    return _gather(res.results, np.asarray(inputs["b_O"], np.float32)), res


def kernel(**inputs):
    out, _ = run(trace=False, **inputs)
    return out
